# revision 1
# baseline (speedup 1.0000x reference)
"""Deformable conv (DFConv2dNoOffset) forward on 8 Trainium2 NeuronCores.

Data-parallel over batch: core b handles image b (8 images, 8 cores).

Per-core algorithm (C=256, H=W=64, K=3, pad=1, dil=1, stride=1):
  out[o, p] = sum_{k, c} W[o, c, k] * S[c, k, p]
  S[c, k, p] = bilinear sample of x[c] at (py, px) = base(p) + tap(k) + off(k, p)
               with zero out-of-bounds contributions (detectron2 semantics).

Bilinear in "difference form" on a zero-padded transposed image P (grid side
GH=68, gy=y+2): with integer cell y0=floor(py) clamped to [-2, 64] and
fy=py-y0 (similarly x):
  S = a + fy*d + fx*h + fy*fx*e
  a[p] = P[p], d[p] = P[p+GH]-P[p], h[p] = P[p+1]-P[p], e[p] = d[p+1]-d[p]
This is algebraically exact vs the reference for every boundary regime
because linear interpolation is continuous and the pad rows are zero.

Pipeline per core:
  prep:  cast x to fp16 into the padded [c, GH*GH] layout, compute d/h/e,
         PE-transpose into a DRAM gather table x4[rows, 4*256] fp16
         (row p = [a | d | h | e], 2KB); pre-transpose weights into
         lhsT[c,o] fp16 tiles; compute per-item (idx, fy, fx, fxy).
  main:  indirect-DMA gather of 2KB rows -> G[128 items, 8, 1024] fp16;
         3 scalar_tensor_tensor FMAs (per-partition scalars) -> S[item, c];
         PE-transpose -> S^T[c, item]; fp16 GEMM, fp32 PSUM accumulation
         over (c-chunk, tap); strided DMA store of out[o, p].
"""

import sys

if "/opt/trn_rl_repo" not in sys.path:
    sys.path.insert(0, "/opt/trn_rl_repo")

import numpy as np

C = 256
H = W = 64
HW = H * W
K2 = 9
GH = 68           # padded grid side (2 + 64 + 2)
GG = GH * GH      # 4624 padded positions
NROW = 4608       # gather-table rows actually written (36*128 >= max idx 4555)
NBLK = HW // 128  # 32 position blocks of 128
NI = NBLK * K2    # 288 item columns in the [128, NI] index/frac layout
O = 256           # output channels
MB = 8            # position blocks per gather op (1024 positions x 1 tap)

_BUILT = None


def _build_kernel():
    from concourse import bacc, mybir, tile
    from concourse.masks import make_identity

    f32 = mybir.dt.float32
    f16 = mybir.dt.float16
    i32 = mybir.dt.int32
    i16 = mybir.dt.int16
    Alu = mybir.AluOpType

    nc = bacc.Bacc("TRN2", target_bir_lowering=False, debug=False,
                   num_swdge_queues=4)

    x_in = nc.dram_tensor("x", [C, HW], f32, kind="ExternalInput")
    off_in = nc.dram_tensor("offset", [2 * K2, HW], f32, kind="ExternalInput")
    w_in = nc.dram_tensor("weight", [O, C * K2], f32, kind="ExternalInput")
    out_ext = nc.dram_tensor("out", [O, HW], f32, kind="ExternalOutput")

    with tile.TileContext(nc) as tc:
        with (
            tc.tile_pool(name="const", bufs=1) as constp,
            tc.tile_pool(name="wpool", bufs=1) as wpool,
            tc.tile_pool(name="scal", bufs=1) as scalp,
            tc.tile_pool(name="dram", bufs=1, space="DRAM") as dramp,
        ):
            x4 = dramp.tile([NROW, 4 * C], f16, name="x4tab")

            ident = constp.tile([128, 128], f16, name="ident16")
            make_identity(nc, ident[:])
            idn18 = constp.tile([2 * K2, 2 * K2], f32, name="idn18")
            make_identity(nc, idn18[:])

            # ---------- phase 0: prep (transient pools) ----------
            with (
                tc.tile_pool(name="prep", bufs=1) as prep,
                tc.tile_pool(name="prep2", bufs=4) as prep2,
                tc.tile_pool(name="psA", bufs=2, space="PSUM") as psA,
            ):
                # ----- padded fp16 image + difference planes, [128c, GG] x2
                comps = {}
                for cc in range(2):
                    xsb = prep.tile([128, HW], f32, name=f"xsb{cc}",
                                    tag="xsb")
                    nc.sync.dma_start(
                        out=xsb[:], in_=x_in[cc * 128:(cc + 1) * 128, :])
                    a = prep.tile([128, GG], f16, name=f"apad{cc}")
                    nc.gpsimd.memset(a[:], 0.0)
                    dst = a[:].rearrange("c (g r) -> c g r", g=GH)[:, 2:2 + H, 2:2 + W]
                    nc.vector.tensor_copy(
                        out=dst,
                        in_=xsb[:].rearrange("c (h w) -> c h w", h=H))
                    d = prep.tile([128, GG], f16, name=f"dpad{cc}")
                    nc.vector.memset(d[:, GG - GH:], 0.0)
                    nc.vector.tensor_tensor(out=d[:, :GG - GH], in0=a[:, GH:],
                                            in1=a[:, :GG - GH], op=Alu.subtract)
                    h = prep.tile([128, GG], f16, name=f"hpad{cc}")
                    nc.vector.memset(h[:, GG - 1:], 0.0)
                    nc.vector.tensor_tensor(out=h[:, :GG - 1], in0=a[:, 1:],
                                            in1=a[:, :GG - 1], op=Alu.subtract)
                    e = prep.tile([128, GG], f16, name=f"epad{cc}")
                    nc.vector.memset(e[:, GG - 1:], 0.0)
                    nc.vector.tensor_tensor(out=e[:, :GG - 1], in0=d[:, 1:],
                                            in1=d[:, :GG - 1], op=Alu.subtract)
                    comps[("a", cc)] = a
                    comps[("d", cc)] = d
                    comps[("h", cc)] = h
                    comps[("e", cc)] = e

                # stage the component planes to DRAM as Z[(comp, c), pos],
                # then xbar-transpose 128-position blocks into x4 rows
                zstage = dramp.tile([4 * C, GG], f16, name="zstage")
                for ci, comp in enumerate(("a", "d", "h", "e")):
                    for cc in range(2):
                        nc.scalar.dma_start(
                            out=zstage[ci * C + cc * 128:
                                       ci * C + cc * 128 + 128, :],
                            in_=comps[(comp, cc)][:])
                for blk in range(NROW // 128):
                    x4blk = prep2.tile([128, 4 * C], f16, tag="x4blk")
                    nc.sync.dma_start_transpose(
                        out=x4blk[:],
                        in_=zstage[:, blk * 128:(blk + 1) * 128])
                    # writes go out on the Activation HWDGE FIFO so they
                    # pipeline against the transposes' SP FIFO
                    nc.scalar.dma_start(out=x4[blk * 128:(blk + 1) * 128, :],
                                        in_=x4blk[:])

                # ----- offsets -> per-item idx / fy / fx / fxy
                dall = prep.tile([2 * K2, HW], f32, name="dall")
                nc.sync.dma_start(out=dall[:], in_=off_in[:, :])

                # offT[128, blk*18 + 2k] = dy_k, ... + 2k+1 = dx_k
                offT = scalp.tile([128, NBLK * 2 * K2], f32, name="offT")
                for blk in range(NBLK):
                    pt18 = psA.tile([128, 2 * K2], f32, tag="offtp")
                    nc.tensor.transpose(
                        out=pt18[:], in_=dall[:, blk * 128:(blk + 1) * 128],
                        identity=idn18[:])
                    nc.scalar.copy(
                        out=offT[:, blk * 2 * K2:(blk + 1) * 2 * K2],
                        in_=pt18[:])
                dyT = offT[:].rearrange("p (b t) -> p b t", t=2 * K2)[
                    :, :, 0:2 * K2:2].rearrange("p b t -> p (b t)")
                dxT = offT[:].rearrange("p (b t) -> p b t", t=2 * K2)[
                    :, :, 1:2 * K2:2].rearrange("p b t -> p (b t)")

                # base coordinates (include the -1 padding shift)
                pidx = prep.tile([128, 1], i32, name="pidx")
                nc.gpsimd.iota(pidx[:], pattern=[[0, 1]], base=0,
                               channel_multiplier=1)
                pidxf = prep.tile([128, 1], f32, name="pidxf")
                nc.vector.tensor_copy(out=pidxf[:], in_=pidx[:])
                geh = prep.tile([128, 1], f32, name="geh")
                nc.vector.tensor_scalar(out=geh[:], in0=pidxf[:], scalar1=63.5,
                                        scalar2=None, op0=Alu.is_gt)
                gehm1 = prep.tile([128, 1], f32, name="gehm1")
                nc.vector.tensor_scalar(out=gehm1[:], in0=geh[:], scalar1=-1.0,
                                        scalar2=None, op0=Alu.add)
                # ybase[p, blk] = 2*blk + (p>=64) - 1
                blk2 = prep.tile([128, NBLK], i32, name="blk2")
                nc.gpsimd.iota(blk2[:], pattern=[[2, NBLK]], base=0,
                               channel_multiplier=0)
                ybase = prep.tile([128, NBLK], f32, name="ybase")
                nc.vector.tensor_copy(out=ybase[:], in_=blk2[:])
                nc.vector.tensor_scalar(out=ybase[:], in0=ybase[:],
                                        scalar1=gehm1[:, 0:1], scalar2=None,
                                        op0=Alu.add)
                # xbase[p] = p - 64*(p>=64) - 1
                xbase = prep.tile([128, 1], f32, name="xbase")
                nc.vector.scalar_tensor_tensor(
                    out=xbase[:], in0=geh[:], scalar=-64.0, in1=pidxf[:],
                    op0=Alu.mult, op1=Alu.add)
                nc.vector.tensor_scalar(out=xbase[:], in0=xbase[:],
                                        scalar1=-1.0, scalar2=None, op0=Alu.add)

                # byk[:, blk*9+k] = ybase[:, blk] + (k//3)
                byk = prep.tile([128, NI], f32, name="byk")
                bxk = prep.tile([128, NI], f32, name="bxk")
                for k in range(K2):
                    nc.vector.tensor_scalar(
                        out=byk[:, k:NI:K2], in0=ybase[:],
                        scalar1=float(k // 3), scalar2=None, op0=Alu.add)
                    nc.vector.tensor_scalar(
                        out=bxk[:, k:NI:K2],
                        in0=xbase[:].broadcast_to([128, NBLK]),
                        scalar1=float(k % 3), scalar2=None, op0=Alu.add)

                py = prep.tile([128, NI], f32, name="py")
                nc.vector.tensor_tensor(out=py[:], in0=byk[:], in1=dyT,
                                        op=Alu.add)
                px = prep.tile([128, NI], f32, name="px")
                nc.vector.tensor_tensor(out=px[:], in0=bxk[:], in1=dxT,
                                        op=Alu.add)

                def floorize(pos, name):
                    """-> (q f32 clamped to [-2,64], frac f32); robust to the
                    f32->i32 rounding mode (trunc or nearest)."""
                    ii = prep.tile([128, NI], i32, name=f"ii_{name}")
                    nc.vector.tensor_copy(out=ii[:], in_=pos[:])
                    ff = prep.tile([128, NI], f32, name=f"ff_{name}")
                    nc.vector.tensor_copy(out=ff[:], in_=ii[:])
                    gt = prep.tile([128, NI], f32, name=f"gt_{name}")
                    nc.vector.tensor_tensor(out=gt[:], in0=ff[:], in1=pos[:],
                                            op=Alu.is_gt)
                    y0 = prep.tile([128, NI], f32, name=f"y0_{name}")
                    nc.vector.tensor_tensor(out=y0[:], in0=ff[:], in1=gt[:],
                                            op=Alu.subtract)
                    fr = prep.tile([128, NI], f32, name=f"fr_{name}")
                    nc.vector.tensor_tensor(out=fr[:], in0=pos[:], in1=y0[:],
                                            op=Alu.subtract)
                    q = prep.tile([128, NI], f32, name=f"q_{name}")
                    nc.vector.tensor_scalar(out=q[:], in0=y0[:], scalar1=-2.0,
                                            scalar2=64.0, op0=Alu.max,
                                            op1=Alu.min)
                    return q, fr

                qy, fyf = floorize(py, "y")
                qx, fxf = floorize(px, "x")

                # idx = (qy+2)*GH + (qx+2) = qy*GH + qx + 2*GH+2
                idxf = prep.tile([128, NI], f32, name="idxf")
                nc.vector.scalar_tensor_tensor(
                    out=idxf[:], in0=qy[:], scalar=float(GH), in1=qx[:],
                    op0=Alu.mult, op1=Alu.add)
                nc.vector.tensor_scalar(out=idxf[:], in0=idxf[:],
                                        scalar1=float(2 * GH + 2),
                                        scalar2=None, op0=Alu.add)
                idx32 = prep.tile([128, NI], i32, name="idx32")
                nc.vector.tensor_copy(out=idx32[:], in_=idxf[:])

                # Build the dma_gather index table: int16, item i of gather op
                # (k, gp) at [i % 16, (k*4+gp)*64 + i // 16], replicated to all
                # 8 Q7-core partition groups.  Item i = m*128 + p samples
                # position (gp*8+m)*128 + p of tap k.
                # 1) free-dim permute to (k, gp, m) order + cast to int16
                idxa = prep.tile([128, NI], i16, name="idxa")
                nc.vector.tensor_copy(
                    out=idxa[:].rearrange("p (k g m) -> p k g m", k=K2, g=4),
                    in_=idx32[:].rearrange("p (g m k) -> p g m k", g=4, m=MB)
                    .transpose([0, 3, 1, 2]))
                # 2) partition fold 128 -> 16: [p = phm*16+j] -> col m*8+phm
                NIG = (NI // MB) * 64  # 2304 columns (36 gather ops x 64)
                idx16 = scalp.tile([128, NIG], i16, name="idx16")
                for phm in range(8):
                    nc.sync.dma_start(
                        out=idx16[0:16, :].rearrange(
                            "j (k g s) -> j k g s", k=K2, g=4)[:, :, :,
                                                              phm:64:MB],
                        in_=idxa[phm * 16:(phm + 1) * 16, :].rearrange(
                            "j (k g m) -> j k g m", k=K2, g=4))
                # 3) replicate partitions 0..15 to the other 7 groups
                for g in range(1, 8):
                    nc.sync.dma_start(out=idx16[g * 16:(g + 1) * 16, :],
                                      in_=idx16[0:16, :])

                fy16 = scalp.tile([128, NI], f16, name="fy16")
                nc.vector.tensor_copy(out=fy16[:], in_=fyf[:])
                fx16 = scalp.tile([128, NI], f16, name="fx16")
                nc.vector.tensor_copy(out=fx16[:], in_=fxf[:])
                fxy16 = scalp.tile([128, NI], f16, name="fxy16")
                nc.vector.tensor_tensor(out=fxy16[:], in0=fy16[:], in1=fx16[:],
                                        op=Alu.mult)

                # ----- weights -> lhsT[c,o] fp16 tiles per (k, cchunk, ochunk)
                wT = {}
                for oc in range(2):
                    wsb32 = prep.tile([128, C * K2], f32, name=f"wsb32{oc}",
                                      tag="wsb32")
                    nc.sync.dma_start(
                        out=wsb32[:], in_=w_in[oc * 128:(oc + 1) * 128, :])
                    wsb = prep.tile([128, C * K2], f16, name=f"wsb{oc}")
                    nc.vector.tensor_copy(out=wsb[:], in_=wsb32[:])
                    for k in range(K2):
                        for cc in range(2):
                            pt = psA.tile([128, 128], f16, tag="wtp")
                            start = cc * 128 * K2 + k
                            src = wsb[:, start: start + 127 * K2 + 1: K2]
                            nc.tensor.transpose(out=pt[:], in_=src,
                                                identity=ident[:])
                            st = wpool.tile([128, 128], f16,
                                            name=f"wT_{k}_{cc}_{oc}")
                            nc.scalar.copy(out=st[:], in_=pt[:])
                            wT[(k, cc, oc)] = st


            # ---------- main pipeline ----------
            with (
                tc.tile_pool(name="gat", bufs=3) as gatp,
                tc.tile_pool(name="spool", bufs=10) as spool,
                tc.tile_pool(name="stpool", bufs=3) as stpool,
                tc.tile_pool(name="pst", bufs=2, space="PSUM") as pst,
                tc.tile_pool(name="psout", bufs=2, space="PSUM") as psout,
                tc.tile_pool(name="outp", bufs=2) as outp,
            ):
                for gp in range(NBLK // MB):   # 4 iterations, 2 pgroups each
                    stiles = {}
                    for k in range(K2):
                        G = gatp.tile([128, MB, 4 * C], f16, tag="G")
                        nc.gpsimd.dma_gather(
                            out_ap=G[:],
                            in_ap=x4[:, :],
                            idxs_ap=idx16[:, (k * 4 + gp) * 64:
                                          (k * 4 + gp) * 64 + 64],
                            num_idxs=MB * 128,
                            num_idxs_reg=MB * 128,
                            elem_size=4 * C,
                            single_packet=False,
                            queue_num=(gp * K2 + k) % 4)
                        for half in range(2):
                            ps = {cc: pst.tile([128, 512], f16, tag=f"stp{cc}",
                                                name=f"stp{cc}")
                                  for cc in range(2)}
                            Ss = []
                            for m4 in range(4):
                                m = half * 4 + m4
                                col = (gp * MB + m) * K2 + k
                                S = spool.tile([128, C], f16, tag="S",
                                               name="S")
                                nc.vector.scalar_tensor_tensor(
                                    out=S[:], in0=G[:, m, C:2 * C],
                                    scalar=fy16[:, col:col + 1],
                                    in1=G[:, m, 0:C],
                                    op0=Alu.mult, op1=Alu.add)
                                nc.vector.scalar_tensor_tensor(
                                    out=S[:], in0=G[:, m, 2 * C:3 * C],
                                    scalar=fx16[:, col:col + 1], in1=S[:],
                                    op0=Alu.mult, op1=Alu.add)
                                nc.vector.scalar_tensor_tensor(
                                    out=S[:], in0=G[:, m, 3 * C:4 * C],
                                    scalar=fxy16[:, col:col + 1], in1=S[:],
                                    op0=Alu.mult, op1=Alu.add)
                                Ss.append(S)
                            for m4 in range(4):
                                for cc in range(2):
                                    nc.tensor.transpose(
                                        out=ps[cc][:, m4 * 128:(m4 + 1) * 128],
                                        in_=Ss[m4][:, cc * 128:(cc + 1) * 128],
                                        identity=ident[:])
                            for cc in range(2):
                                st = stpool.tile([128, 512], f16,
                                                 tag=f"st_{k}_{cc}", bufs=3)
                                nc.scalar.copy(out=st[:], in_=ps[cc][:])
                                stiles[(half, k, cc)] = st

                    for half in range(2):
                        pg = gp * 2 + half
                        for oc in range(2):
                            acc = psout.tile([128, 512], f32, tag="acc")
                            first = True
                            for k in range(K2):
                                for cc in range(2):
                                    nc.tensor.matmul(
                                        out=acc[:],
                                        lhsT=wT[(k, cc, oc)][:],
                                        rhs=stiles[(half, k, cc)][:],
                                        start=first,
                                        stop=(k == K2 - 1 and cc == 1))
                                    first = False
                            osb = outp.tile([128, 512], f32, tag="osb")
                            nc.scalar.copy(out=osb[:], in_=acc[:])
                            nc.sync.dma_start(
                                out=out_ext[oc * 128:(oc + 1) * 128,
                                            pg * 512:(pg + 1) * 512],
                                in_=osb[:])

    nc.compile()
    return nc


def kernel(x, offset, weight):
    global _BUILT
    from concourse import bass_utils

    if _BUILT is None:
        _BUILT = _build_kernel()
    nc = _BUILT

    B = x.shape[0]
    x = np.ascontiguousarray(np.asarray(x, np.float32).reshape(B, C, HW))
    offset = np.ascontiguousarray(
        np.asarray(offset, np.float32).reshape(B, 2 * K2, HW))
    weight = np.ascontiguousarray(
        np.asarray(weight, np.float32).reshape(O, C * K2))

    in_maps = [{"x": x[b], "offset": offset[b], "weight": weight}
               for b in range(B)]
    res = bass_utils.run_bass_kernel_spmd(nc, in_maps, core_ids=list(range(B)))
    outs = [np.asarray(res.results[b]["out"]).reshape(O, H, W)
            for b in range(B)]
    return np.stack(outs).astype(np.float32)



# revision 3
# speedup vs baseline: 1.2597x; 1.2597x over previous
"""Deformable conv (DFConv2dNoOffset) forward on 8 Trainium2 NeuronCores.

Data-parallel over batch: core b handles image b (8 images, 8 cores).

Per-core algorithm (C=256, H=W=64, K=3, pad=1, dil=1, stride=1):
  out[o, p] = sum_{k, c} W[o, c, k] * S[c, k, p]
  S[c, k, p] = bilinear sample of x[c] at (py, px) = base(p) + tap(k) + off(k, p)
               with zero out-of-bounds contributions (detectron2 semantics).

Bilinear in "difference form" on a zero-padded transposed image P (grid side
GH=68, gy=y+2): with integer cell y0=floor(py) clamped to [-2, 64] and
fy=py-y0 (similarly x):
  S = (a + fy*d) + fx*(h + fy*e)
  a[g] = P[g], d[g] = P[g+GH]-P[g], h[g] = P[g+1]-P[g], e[g] = d[g+1]-d[g]
This is algebraically exact vs the reference for every boundary regime
because linear interpolation is continuous and the pad rows are zero.

Pipeline per core (v2):
  prep:  cast x to bf16 into the padded [c, GH*GH] layout, compute d/h/e on
         DVE; build the DRAM gather table x4[row g, 1024] (= [a|h|d|e] x 256c
         bf16, 2KB rows) with PE transposes + PSUM->SBUF copies + large
         contiguous HWDGE writes (the v1 DMA-transpose path cost ~300us in
         tiny descriptors); concurrently compute per-item (idx, fy, fx) and
         pre-transpose weights into lhsT[c,o] bf16 tiles.
  main:  36x SWDGE dma_gather of 2KB rows -> G[128 items, 8, 1024] bf16;
         2 scalar_tensor_tensor FMAs per (m, tap):
             UV = [a|h] + fy*[d|e]   (512 free)
             S  = u + fx*v           (256 free)
         (bf16 sources keep the DVE STT fast path); PE-transpose ->
         S^T[c, item]; bf16 GEMM with fp32 PSUM accumulation over
         (c-chunk, tap); strided DMA store of out[o, p].
"""

import sys

if "/opt/trn_rl_repo" not in sys.path:
    sys.path.insert(0, "/opt/trn_rl_repo")

import numpy as np

C = 256
H = W = 64
HW = H * W
K2 = 9
GH = 68           # padded grid side (2 + 64 + 2)
GG = GH * GH      # 4624 padded positions
NROW = 4608       # gather-table rows actually written (36*128 >= max idx 4555)
NBLK = HW // 128  # 32 position blocks of 128
NI = NBLK * K2    # 288 item columns in the [128, NI] index/frac layout
O = 256           # output channels
MB = 8            # position blocks per gather op (1024 positions x 1 tap)

_BUILT = None


def _build_kernel():
    from concourse import bacc, mybir, tile
    from concourse.masks import make_identity

    f32 = mybir.dt.float32
    bf16 = mybir.dt.bfloat16
    i32 = mybir.dt.int32
    i16 = mybir.dt.int16
    Alu = mybir.AluOpType

    nc = bacc.Bacc("TRN2", target_bir_lowering=False, debug=False,
                   num_swdge_queues=4)

    x_in = nc.dram_tensor("x", [C, HW], f32, kind="ExternalInput")
    off_in = nc.dram_tensor("offset", [2 * K2, HW], f32, kind="ExternalInput")
    w_in = nc.dram_tensor("weight", [O, C * K2], f32, kind="ExternalInput")
    out_ext = nc.dram_tensor("out", [O, HW], f32, kind="ExternalOutput")

    with tile.TileContext(nc) as tc:
        with (
            tc.tile_pool(name="const", bufs=1) as constp,
            tc.tile_pool(name="wpool", bufs=1) as wpool,
            tc.tile_pool(name="scal", bufs=1) as scalp,
            tc.tile_pool(name="dram", bufs=1, space="DRAM") as dramp,
        ):
            x4 = dramp.tile([NROW, 4 * C], bf16, name="x4tab")

            ident = constp.tile([128, 128], bf16, name="identb")
            make_identity(nc, ident[:])
            idn18 = constp.tile([2 * K2, 2 * K2], f32, name="idn18")
            make_identity(nc, idn18[:])

            # ---------- phase 0: prep (transient pools) ----------
            with (
                tc.tile_pool(name="prep", bufs=1) as prep,
                tc.tile_pool(name="stgp", bufs=4) as stgp,
                tc.tile_pool(name="psA", bufs=2, space="PSUM") as psA,
                tc.tile_pool(name="psB", bufs=4, space="PSUM") as psB,
            ):
                # ----- padded bf16 image + difference planes, [128c, GG] x2
                # component order in the gather token: [a | h | d | e]
                comps = {}
                for cc in range(2):
                    xsb = prep.tile([128, HW], f32, name=f"xsb{cc}",
                                    tag="xsb")
                    nc.sync.dma_start(
                        out=xsb[:], in_=x_in[cc * 128:(cc + 1) * 128, :])
                    a = prep.tile([128, GG], bf16, name=f"apad{cc}")
                    nc.gpsimd.memset(a[:], 0.0)
                    dst = a[:].rearrange("c (g r) -> c g r", g=GH)[:, 2:2 + H, 2:2 + W]
                    nc.vector.tensor_copy(
                        out=dst,
                        in_=xsb[:].rearrange("c (h w) -> c h w", h=H))
                    d = prep.tile([128, GG], bf16, name=f"dpad{cc}")
                    nc.vector.memset(d[:, GG - GH:], 0.0)
                    nc.vector.tensor_tensor(out=d[:, :GG - GH], in0=a[:, GH:],
                                            in1=a[:, :GG - GH], op=Alu.subtract)
                    h = prep.tile([128, GG], bf16, name=f"hpad{cc}")
                    nc.vector.memset(h[:, GG - 1:], 0.0)
                    nc.vector.tensor_tensor(out=h[:, :GG - 1], in0=a[:, 1:],
                                            in1=a[:, :GG - 1], op=Alu.subtract)
                    e = prep.tile([128, GG], bf16, name=f"epad{cc}")
                    nc.vector.memset(e[:, GG - 1:], 0.0)
                    nc.vector.tensor_tensor(out=e[:, :GG - 1], in0=d[:, 1:],
                                            in1=d[:, :GG - 1], op=Alu.subtract)
                    comps[("a", cc)] = a
                    comps[("d", cc)] = d
                    comps[("h", cc)] = h
                    comps[("e", cc)] = e

                # ----- gather-table build: PE-transpose 128-cell blocks of
                # each component plane into x4 rows [g, a|h|d|e], then one
                # large contiguous HWDGE write per block.
                ORDER = ("a", "h", "d", "e")
                for blk in range(NROW // 128):
                    stg = stgp.tile([128, 4 * C], bf16, tag="stg")
                    for grp in range(2):  # grp0 = [a|h], grp1 = [d|e]
                        ps = psB.tile([128, 512], bf16, tag="pstg")
                        for ci in range(2):
                            comp = ORDER[grp * 2 + ci]
                            for cc in range(2):
                                nc.tensor.transpose(
                                    out=ps[:, (ci * 2 + cc) * 128:
                                            (ci * 2 + cc) * 128 + 128],
                                    in_=comps[(comp, cc)][
                                        :, blk * 128:(blk + 1) * 128],
                                    identity=ident[:])
                        if grp == 0:
                            nc.scalar.copy(
                                out=stg[:, grp * 512:(grp + 1) * 512],
                                in_=ps[:])
                        else:
                            nc.vector.tensor_copy(
                                out=stg[:, grp * 512:(grp + 1) * 512],
                                in_=ps[:])
                    nc.scalar.dma_start(out=x4[blk * 128:(blk + 1) * 128, :],
                                        in_=stg[:])

                # ----- offsets -> per-item idx / fy / fx
                dall = prep.tile([2 * K2, HW], f32, name="dall")
                nc.sync.dma_start(out=dall[:], in_=off_in[:, :])

                # offT[128, blk*18 + 2k] = dy_k, ... + 2k+1 = dx_k
                offT = scalp.tile([128, NBLK * 2 * K2], f32, name="offT")
                for blk in range(NBLK):
                    pt18 = psA.tile([128, 2 * K2], f32, tag="offtp")
                    nc.tensor.transpose(
                        out=pt18[:], in_=dall[:, blk * 128:(blk + 1) * 128],
                        identity=idn18[:])
                    nc.scalar.copy(
                        out=offT[:, blk * 2 * K2:(blk + 1) * 2 * K2],
                        in_=pt18[:])
                dyT = offT[:].rearrange("p (b t) -> p b t", t=2 * K2)[
                    :, :, 0:2 * K2:2].rearrange("p b t -> p (b t)")
                dxT = offT[:].rearrange("p (b t) -> p b t", t=2 * K2)[
                    :, :, 1:2 * K2:2].rearrange("p b t -> p (b t)")

                # base coordinates (include the -1 padding shift)
                pidx = prep.tile([128, 1], i32, name="pidx")
                nc.gpsimd.iota(pidx[:], pattern=[[0, 1]], base=0,
                               channel_multiplier=1)
                pidxf = prep.tile([128, 1], f32, name="pidxf")
                nc.vector.tensor_copy(out=pidxf[:], in_=pidx[:])
                geh = prep.tile([128, 1], f32, name="geh")
                nc.vector.tensor_scalar(out=geh[:], in0=pidxf[:], scalar1=63.5,
                                        scalar2=None, op0=Alu.is_gt)
                gehm1 = prep.tile([128, 1], f32, name="gehm1")
                nc.vector.tensor_scalar(out=gehm1[:], in0=geh[:], scalar1=-1.0,
                                        scalar2=None, op0=Alu.add)
                # ybase[p, blk] = 2*blk + (p>=64) - 1
                blk2 = prep.tile([128, NBLK], i32, name="blk2")
                nc.gpsimd.iota(blk2[:], pattern=[[2, NBLK]], base=0,
                               channel_multiplier=0)
                ybase = prep.tile([128, NBLK], f32, name="ybase")
                nc.vector.tensor_copy(out=ybase[:], in_=blk2[:])
                nc.vector.tensor_scalar(out=ybase[:], in0=ybase[:],
                                        scalar1=gehm1[:, 0:1], scalar2=None,
                                        op0=Alu.add)
                # xbase[p] = p - 64*(p>=64) - 1
                xbase = prep.tile([128, 1], f32, name="xbase")
                nc.vector.scalar_tensor_tensor(
                    out=xbase[:], in0=geh[:], scalar=-64.0, in1=pidxf[:],
                    op0=Alu.mult, op1=Alu.add)
                nc.vector.tensor_scalar(out=xbase[:], in0=xbase[:],
                                        scalar1=-1.0, scalar2=None, op0=Alu.add)

                # byk[:, blk*9+k] = ybase[:, blk] + (k//3)
                byk = prep.tile([128, NI], f32, name="byk")
                bxk = prep.tile([128, NI], f32, name="bxk")
                for k in range(K2):
                    nc.vector.tensor_scalar(
                        out=byk[:, k:NI:K2], in0=ybase[:],
                        scalar1=float(k // 3), scalar2=None, op0=Alu.add)
                    nc.vector.tensor_scalar(
                        out=bxk[:, k:NI:K2],
                        in0=xbase[:].broadcast_to([128, NBLK]),
                        scalar1=float(k % 3), scalar2=None, op0=Alu.add)

                py = prep.tile([128, NI], f32, name="py")
                nc.vector.tensor_tensor(out=py[:], in0=byk[:], in1=dyT,
                                        op=Alu.add)
                px = prep.tile([128, NI], f32, name="px")
                nc.vector.tensor_tensor(out=px[:], in0=bxk[:], in1=dxT,
                                        op=Alu.add)

                def floorize(pos, name):
                    """-> (q f32 clamped to [-2,64], frac f32); robust to the
                    f32->i32 rounding mode (trunc or nearest)."""
                    ii = prep.tile([128, NI], i32, name=f"ii_{name}")
                    nc.vector.tensor_copy(out=ii[:], in_=pos[:])
                    ff = prep.tile([128, NI], f32, name=f"ff_{name}")
                    nc.vector.tensor_copy(out=ff[:], in_=ii[:])
                    gt = prep.tile([128, NI], f32, name=f"gt_{name}")
                    nc.vector.tensor_tensor(out=gt[:], in0=ff[:], in1=pos[:],
                                            op=Alu.is_gt)
                    y0 = prep.tile([128, NI], f32, name=f"y0_{name}")
                    nc.vector.tensor_tensor(out=y0[:], in0=ff[:], in1=gt[:],
                                            op=Alu.subtract)
                    fr = prep.tile([128, NI], f32, name=f"fr_{name}")
                    nc.vector.tensor_tensor(out=fr[:], in0=pos[:], in1=y0[:],
                                            op=Alu.subtract)
                    q = prep.tile([128, NI], f32, name=f"q_{name}")
                    nc.vector.tensor_scalar(out=q[:], in0=y0[:], scalar1=-2.0,
                                            scalar2=64.0, op0=Alu.max,
                                            op1=Alu.min)
                    return q, fr

                qy, fyf = floorize(py, "y")
                qx, fxf = floorize(px, "x")

                # idx = (qy+2)*GH + (qx+2) = qy*GH + qx + 2*GH+2
                idxf = prep.tile([128, NI], f32, name="idxf")
                nc.vector.scalar_tensor_tensor(
                    out=idxf[:], in0=qy[:], scalar=float(GH), in1=qx[:],
                    op0=Alu.mult, op1=Alu.add)
                nc.vector.tensor_scalar(out=idxf[:], in0=idxf[:],
                                        scalar1=float(2 * GH + 2),
                                        scalar2=None, op0=Alu.add)
                idx32 = prep.tile([128, NI], i32, name="idx32")
                nc.vector.tensor_copy(out=idx32[:], in_=idxf[:])

                # Build the dma_gather index table: int16, item i of gather op
                # (k, gp) at [i % 16, (k*4+gp)*64 + i // 16], replicated to all
                # 8 Q7-core partition groups.  Item i = m*128 + p samples
                # position (gp*8+m)*128 + p of tap k.
                # 1) free-dim permute to (k, gp, m) order + cast to int16
                idxa = prep.tile([128, NI], i16, name="idxa")
                nc.vector.tensor_copy(
                    out=idxa[:].rearrange("p (k g m) -> p k g m", k=K2, g=4),
                    in_=idx32[:].rearrange("p (g m k) -> p g m k", g=4, m=MB)
                    .transpose([0, 3, 1, 2]))
                # 2) partition fold 128 -> 16: [p = phm*16+j] -> col m*8+phm
                NIG = (NI // MB) * 64  # 2304 columns (36 gather ops x 64)
                idx16 = scalp.tile([128, NIG], i16, name="idx16")
                for phm in range(8):
                    nc.sync.dma_start(
                        out=idx16[0:16, :].rearrange(
                            "j (k g s) -> j k g s", k=K2, g=4)[:, :, :,
                                                              phm:64:MB],
                        in_=idxa[phm * 16:(phm + 1) * 16, :].rearrange(
                            "j (k g m) -> j k g m", k=K2, g=4))
                # 3) replicate partitions 0..15 to the other 7 groups
                for g in range(1, 8):
                    nc.sync.dma_start(out=idx16[g * 16:(g + 1) * 16, :],
                                      in_=idx16[0:16, :])

                fy16 = scalp.tile([128, NI], bf16, name="fy16")
                nc.vector.tensor_copy(out=fy16[:], in_=fyf[:])
                fx16 = scalp.tile([128, NI], bf16, name="fx16")
                nc.vector.tensor_copy(out=fx16[:], in_=fxf[:])

                # ----- weights -> lhsT[c,o] bf16 tiles per (k, cchunk, ochunk)
                wT = {}
                for oc in range(2):
                    wsb32 = prep.tile([128, C * K2], f32, name=f"wsb32{oc}",
                                      tag="wsb32")
                    nc.sync.dma_start(
                        out=wsb32[:], in_=w_in[oc * 128:(oc + 1) * 128, :])
                    wsb = prep.tile([128, C * K2], bf16, name=f"wsb{oc}")
                    nc.vector.tensor_copy(out=wsb[:], in_=wsb32[:])
                    for k in range(K2):
                        for cc in range(2):
                            pt = psA.tile([128, 128], bf16, tag="wtp")
                            start = cc * 128 * K2 + k
                            src = wsb[:, start: start + 127 * K2 + 1: K2]
                            nc.tensor.transpose(out=pt[:], in_=src,
                                                identity=ident[:])
                            st = wpool.tile([128, 128], bf16,
                                            name=f"wT_{k}_{cc}_{oc}")
                            nc.scalar.copy(out=st[:], in_=pt[:])
                            wT[(k, cc, oc)] = st


            # ---------- main pipeline ----------
            with (
                tc.tile_pool(name="gat", bufs=4) as gatp,
                tc.tile_pool(name="spool", bufs=10) as spool,
                tc.tile_pool(name="uvpool", bufs=10) as uvpool,
                tc.tile_pool(name="stpool", bufs=3) as stpool,
                tc.tile_pool(name="pst", bufs=2, space="PSUM") as pst,
                tc.tile_pool(name="psout", bufs=2, space="PSUM") as psout,
                tc.tile_pool(name="outp", bufs=2) as outp,
            ):
                for gp in range(NBLK // MB):   # 4 iterations, 2 pgroups each
                    stiles = {}
                    for k in range(K2):
                        G = gatp.tile([128, MB, 4 * C], bf16, tag="G")
                        nc.gpsimd.dma_gather(
                            out_ap=G[:],
                            in_ap=x4[:, :],
                            idxs_ap=idx16[:, (k * 4 + gp) * 64:
                                          (k * 4 + gp) * 64 + 64],
                            num_idxs=MB * 128,
                            num_idxs_reg=MB * 128,
                            elem_size=4 * C,
                            single_packet=False,
                            queue_num=(gp * K2 + k) % 4)
                        for half in range(2):
                            ps = {cc: pst.tile([128, 512], bf16,
                                               tag=f"stp{cc}", name=f"stp{cc}")
                                  for cc in range(2)}
                            Ss = []
                            for m4 in range(4):
                                m = half * 4 + m4
                                col = (gp * MB + m) * K2 + k
                                # UV = [a|h] + fy * [d|e]   (free dim 512)
                                UV = uvpool.tile([128, 2 * C], bf16, tag="UV",
                                                 name="UV")
                                nc.vector.scalar_tensor_tensor(
                                    out=UV[:], in0=G[:, m, 2 * C:4 * C],
                                    scalar=fy16[:, col:col + 1],
                                    in1=G[:, m, 0:2 * C],
                                    op0=Alu.mult, op1=Alu.add)
                                # S = u + fx * v            (free dim 256)
                                S = spool.tile([128, C], bf16, tag="S",
                                               name="S")
                                nc.vector.scalar_tensor_tensor(
                                    out=S[:], in0=UV[:, C:2 * C],
                                    scalar=fx16[:, col:col + 1],
                                    in1=UV[:, 0:C],
                                    op0=Alu.mult, op1=Alu.add)
                                Ss.append(S)
                            for m4 in range(4):
                                for cc in range(2):
                                    nc.tensor.transpose(
                                        out=ps[cc][:, m4 * 128:(m4 + 1) * 128],
                                        in_=Ss[m4][:, cc * 128:(cc + 1) * 128],
                                        identity=ident[:])
                            for cc in range(2):
                                st = stpool.tile([128, 512], bf16,
                                                 tag=f"st_{k}_{cc}", bufs=3)
                                nc.scalar.copy(out=st[:], in_=ps[cc][:])
                                stiles[(half, k, cc)] = st

                    for half in range(2):
                        pg = gp * 2 + half
                        for oc in range(2):
                            acc = psout.tile([128, 512], f32, tag="acc")
                            first = True
                            for k in range(K2):
                                for cc in range(2):
                                    nc.tensor.matmul(
                                        out=acc[:],
                                        lhsT=wT[(k, cc, oc)][:],
                                        rhs=stiles[(half, k, cc)][:],
                                        start=first,
                                        stop=(k == K2 - 1 and cc == 1))
                                    first = False
                            osb = outp.tile([128, 512], f32, tag="osb")
                            nc.scalar.copy(out=osb[:], in_=acc[:])
                            nc.sync.dma_start(
                                out=out_ext[oc * 128:(oc + 1) * 128,
                                            pg * 512:(pg + 1) * 512],
                                in_=osb[:])

    nc.compile()
    return nc


def kernel(x, offset, weight):
    global _BUILT
    from concourse import bass_utils

    if _BUILT is None:
        _BUILT = _build_kernel()
    nc = _BUILT

    B = x.shape[0]
    x = np.ascontiguousarray(np.asarray(x, np.float32).reshape(B, C, HW))
    offset = np.ascontiguousarray(
        np.asarray(offset, np.float32).reshape(B, 2 * K2, HW))
    weight = np.ascontiguousarray(
        np.asarray(weight, np.float32).reshape(O, C * K2))

    in_maps = [{"x": x[b], "offset": offset[b], "weight": weight}
               for b in range(B)]
    res = bass_utils.run_bass_kernel_spmd(nc, in_maps, core_ids=list(range(B)))
    outs = [np.asarray(res.results[b]["out"]).reshape(O, H, W)
            for b in range(B)]
    return np.stack(outs).astype(np.float32)


# revision 12
# speedup vs baseline: 1.4571x; 1.1567x over previous
"""Deformable conv (DFConv2dNoOffset) forward on 8 Trainium2 NeuronCores.

Data-parallel over batch: core b handles image b (8 images, 8 cores).

Per-core algorithm (C=256, H=W=64, K=3, pad=1, dil=1, stride=1):
  out[o, p] = sum_{k, c} W[o, c, k] * S[c, k, p]
  S[c, k, p] = bilinear sample of x[c] at (py, px) = base(p) + tap(k) + off(k, p)
               with zero out-of-bounds contributions (detectron2 semantics).

Bilinear in "difference form" on a zero-padded transposed image P (grid side
GH=68, gy=y+2): with integer cell y0=floor(py) clamped to [-2, 64] and
fy=py-y0 (similarly x):
  S = (a + fy*d) + fx*(h + fy*e)
  a[g] = P[g], d[g] = P[g+GH]-P[g], h[g] = P[g+1]-P[g], e[g] = d[g+1]-d[g]
This is algebraically exact vs the reference for every boundary regime
because linear interpolation is continuous and the pad rows are zero.

Pipeline per core (v3):
  prep:  cast x to bf16 into the padded [c, GH*GH] layout, compute d/h/e on
         DVE; build the DRAM gather table x4[row g, 1024] (= [a|h|d|e] x 256c
         bf16, 2KB rows) with PE transposes + PSUM->SBUF copies + large
         contiguous HWDGE writes.  The dma_gather index table needs int16
         indices in a [16, cols] wrapped layout; computing it in the natural
         [128, NI] layout requires a 128->16 partition fold that costs ~90us
         in 2-byte DMA descriptors, so the index pipeline instead runs
         directly in [16, 2304] layout fed by 256 PE mini-transposes of the
         offsets; a separate cheap [128, NI] pipeline produces the fy/fx
         per-partition STT scalars.  Weights are pre-transposed into lhsT
         bf16 tiles.
  main:  36x SWDGE dma_gather of 2KB rows -> G[128 items, 8, 1024] bf16;
         2 scalar_tensor_tensor FMAs per (m, tap):
             UV = [a|h] + fy*[d|e]   (512 free)
             S  = u + fx*v           (256 free)
         PE-transpose -> S^T[c, item]; bf16 GEMM with fp32 PSUM accumulation
         over (c-chunk, tap); strided DMA store of out[o, p].
"""

import sys

if "/opt/trn_rl_repo" not in sys.path:
    sys.path.insert(0, "/opt/trn_rl_repo")

import numpy as np

C = 256
H = W = 64
HW = H * W
K2 = 9
GH = 68           # padded grid side (2 + 64 + 2)
GG = GH * GH      # 4624 padded positions
NROW = 4608       # gather-table rows actually written (36*128 >= max idx 4555)
NBLK = HW // 128  # 32 position blocks of 128
NI = NBLK * K2    # 288 item columns in the [128, NI] index/frac layout
NI16 = NBLK * 8 * K2  # 2304 item columns in the [16, NI16] idx layout
O = 256           # output channels
MB = 8            # position blocks per gather op (1024 positions x 1 tap)

_BUILT = None


def _build_kernel():
    from concourse import bacc, mybir, tile
    from concourse.masks import make_identity

    f32 = mybir.dt.float32
    bf16 = mybir.dt.bfloat16
    i32 = mybir.dt.int32
    i16 = mybir.dt.int16
    Alu = mybir.AluOpType

    nc = bacc.Bacc("TRN2", target_bir_lowering=False, debug=False,
                   num_swdge_queues=4)

    x_in = nc.dram_tensor("x", [C, HW], f32, kind="ExternalInput")
    off_in = nc.dram_tensor("offset", [2 * K2, HW], f32, kind="ExternalInput")
    w_in = nc.dram_tensor("weight", [O, C * K2], f32, kind="ExternalInput")
    out_ext = nc.dram_tensor("out", [O, HW], f32, kind="ExternalOutput")

    with tile.TileContext(nc) as tc:
        with (
            tc.tile_pool(name="const", bufs=1) as constp,
            tc.tile_pool(name="wpool", bufs=1) as wpool,
            tc.tile_pool(name="scal", bufs=1) as scalp,
            tc.tile_pool(name="dram", bufs=1, space="DRAM") as dramp,
        ):
            x4 = dramp.tile([NROW, 4 * C], bf16, name="x4tab")

            ident = constp.tile([128, 128], bf16, name="identb")
            make_identity(nc, ident[:])
            idn18 = constp.tile([2 * K2, 2 * K2], f32, name="idn18")
            make_identity(nc, idn18[:])

            # ---------- phase 0: prep (transient pools) ----------
            with (
                tc.tile_pool(name="prep", bufs=1) as prep,
                tc.tile_pool(name="stgp", bufs=4) as stgp,
                tc.tile_pool(name="psA", bufs=2, space="PSUM") as psA,
                tc.tile_pool(name="psB", bufs=2, space="PSUM") as psB,
                tc.tile_pool(name="ps16", bufs=2, space="PSUM") as ps16p,
            ):
                # ----- input loads (issue all up front).  x and w are cast
                # f32 -> bf16 during the DMA (SWDGE cast path), straight into
                # their final layouts -- no f32 staging tiles.
                dall = prep.tile([2 * K2, HW], f32, name="dall")
                nc.sync.dma_start(out=dall[:], in_=off_in[:, :])

                # ----- offsets: 256 mini PE transposes into [16, (b,phm,18)]
                offT16 = scalp.tile([16, NBLK * 8 * 18], f32, name="offT16")
                for b in range(NBLK):
                    pt = ps16p.tile([16, 8 * 18], f32, tag="pt16")
                    for phm in range(8):
                        nc.tensor.transpose(
                            out=pt[:, phm * 18:(phm + 1) * 18],
                            in_=dall[:, b * 128 + phm * 16:
                                     b * 128 + (phm + 1) * 16],
                            identity=idn18[:])
                    nc.scalar.copy(
                        out=offT16[:, b * 144:(b + 1) * 144], in_=pt[:])

                # offT[128, blk*18 + 2k] for the fy/fx scalar pipeline
                offT = scalp.tile([128, NBLK * 2 * K2], f32, name="offT")
                for blk in range(NBLK):
                    pt18 = psA.tile([128, 2 * K2], f32, tag="offtp")
                    nc.tensor.transpose(
                        out=pt18[:], in_=dall[:, blk * 128:(blk + 1) * 128],
                        identity=idn18[:])
                    nc.scalar.copy(
                        out=offT[:, blk * 2 * K2:(blk + 1) * 2 * K2],
                        in_=pt18[:])

                # ----- padded bf16 image + difference planes, [128c, GG] x2
                # component order in the gather token: [a | h | d | e]
                comps = {}
                for cc in range(2):
                    a = prep.tile([128, GG], bf16, name=f"apad{cc}")
                    nc.gpsimd.memset(a[:], 0.0)
                    dst = a[:].rearrange("c (g r) -> c g r", g=GH)[:, 2:2 + H, 2:2 + W]
                    nc.gpsimd.dma_start(
                        out=dst,
                        in_=x_in[cc * 128:(cc + 1) * 128, :].rearrange(
                            "c (h w) -> c h w", h=H))
                    d = prep.tile([128, GG], bf16, name=f"dpad{cc}")
                    nc.vector.memset(d[:, GG - GH:], 0.0)
                    nc.vector.tensor_tensor(out=d[:, :GG - GH], in0=a[:, GH:],
                                            in1=a[:, :GG - GH], op=Alu.subtract)
                    h = prep.tile([128, GG], bf16, name=f"hpad{cc}")
                    nc.vector.memset(h[:, GG - 1:], 0.0)
                    nc.vector.tensor_tensor(out=h[:, :GG - 1], in0=a[:, 1:],
                                            in1=a[:, :GG - 1], op=Alu.subtract)
                    e = prep.tile([128, GG], bf16, name=f"epad{cc}")
                    nc.vector.memset(e[:, GG - 1:], 0.0)
                    nc.vector.tensor_tensor(out=e[:, :GG - 1], in0=d[:, 1:],
                                            in1=d[:, :GG - 1], op=Alu.subtract)
                    comps[("a", cc)] = a
                    comps[("d", cc)] = d
                    comps[("h", cc)] = h
                    comps[("e", cc)] = e

                # ----- [16, NI16] idx pipeline: cols (b, phm, k).  Uses 5
                # working tiles (tA i32 + tB/tC/tD/tE f32) rewritten in
                # place to keep the prep pool small.
                # by16 = 2b + (phm>=4) + k//3 - 1 ; bx16 = 16*(phm%4)+j+(k%3)-1
                tA = prep.tile([16, NI16], i32, name="tA16")
                tB = prep.tile([16, NI16], f32, name="tB16")
                tC = prep.tile([16, NI16], f32, name="tC16")
                tD = prep.tile([16, NI16], f32, name="tD16")
                tE = prep.tile([16, NI16], f32, name="tE16")
                o16v = offT16[:].rearrange("j (bp r) -> j bp r", r=18)
                dy16 = o16v[:, :, 0:18:2].rearrange("j bp k -> j (bp k)")
                dx16 = o16v[:, :, 1:18:2].rearrange("j bp k -> j (bp k)")
                # tB = py16, tC = px16  (iota patterns are limited to 4 dims,
                # so each base grid is built from two iotas + an add)
                nc.gpsimd.iota(tA[:],
                               pattern=[[2, NBLK], [1, 2], [0, 36]],
                               base=-1, channel_multiplier=0)
                nc.vector.tensor_copy(out=tB[:], in_=tA[:])
                nc.gpsimd.iota(tA[:],
                               pattern=[[0, 256], [1, 3], [0, 3]],
                               base=0, channel_multiplier=0)
                nc.vector.tensor_copy(out=tE[:], in_=tA[:])
                nc.vector.tensor_tensor(out=tB[:], in0=tB[:], in1=tE[:],
                                        op=Alu.add)
                nc.vector.tensor_tensor(out=tB[:], in0=tB[:], in1=dy16,
                                        op=Alu.add)
                nc.gpsimd.iota(tA[:],
                               pattern=[[0, 64], [16, 4], [0, 9]],
                               base=-1, channel_multiplier=1)
                nc.vector.tensor_copy(out=tC[:], in_=tA[:])
                nc.gpsimd.iota(tA[:],
                               pattern=[[0, 256], [0, 3], [1, 3]],
                               base=0, channel_multiplier=0)
                nc.vector.tensor_copy(out=tE[:], in_=tA[:])
                nc.vector.tensor_tensor(out=tC[:], in0=tC[:], in1=tE[:],
                                        op=Alu.add)
                nc.vector.tensor_tensor(out=tC[:], in0=tC[:], in1=dx16,
                                        op=Alu.add)
                # qy16 = clamp(floor(tB)) -> tD  (robust to f32->i32 rounding)
                nc.vector.tensor_copy(out=tA[:], in_=tB[:])
                nc.vector.tensor_copy(out=tD[:], in_=tA[:])
                nc.vector.tensor_tensor(out=tE[:], in0=tD[:], in1=tB[:],
                                        op=Alu.is_gt)
                nc.vector.tensor_tensor(out=tD[:], in0=tD[:], in1=tE[:],
                                        op=Alu.subtract)
                nc.vector.tensor_scalar(out=tD[:], in0=tD[:], scalar1=-2.0,
                                        scalar2=64.0, op0=Alu.max,
                                        op1=Alu.min)
                # qx16 = clamp(floor(tC)) -> tB
                nc.vector.tensor_copy(out=tA[:], in_=tC[:])
                nc.vector.tensor_copy(out=tB[:], in_=tA[:])
                nc.vector.tensor_tensor(out=tE[:], in0=tB[:], in1=tC[:],
                                        op=Alu.is_gt)
                nc.vector.tensor_tensor(out=tB[:], in0=tB[:], in1=tE[:],
                                        op=Alu.subtract)
                nc.vector.tensor_scalar(out=tB[:], in0=tB[:], scalar1=-2.0,
                                        scalar2=64.0, op0=Alu.max,
                                        op1=Alu.min)
                # idx = qy*GH + qx + (2*GH+2) -> tC -> tA (i32)
                nc.vector.scalar_tensor_tensor(
                    out=tC[:], in0=tD[:], scalar=float(GH), in1=tB[:],
                    op0=Alu.mult, op1=Alu.add)
                nc.vector.tensor_scalar(out=tC[:], in0=tC[:],
                                        scalar1=float(2 * GH + 2),
                                        scalar2=None, op0=Alu.add)
                nc.vector.tensor_copy(out=tA[:], in_=tC[:])

                # idx table [16, (k, gp, m, phm)] int16 + replicate to the
                # other 7 Q7-core partition groups (contiguous 4.6KB DMAs)
                idx16 = scalp.tile([128, NI16], i16, name="idx16")
                nc.vector.tensor_copy(
                    out=idx16[0:16, :].rearrange(
                        "j (k b q) -> j k b q", k=K2, b=NBLK),
                    in_=tA[:].rearrange(
                        "j (b q k) -> j b q k", b=NBLK, q=8)
                    .transpose([0, 3, 1, 2]))
                for g in range(1, 8):
                    nc.sync.dma_start(out=idx16[g * 16:(g + 1) * 16, :],
                                      in_=idx16[0:16, :])

                # ----- [128, NI] pipeline for the fy/fx per-partition scalars
                dyT = offT[:].rearrange("p (b t) -> p b t", t=2 * K2)[
                    :, :, 0:2 * K2:2].rearrange("p b t -> p (b t)")
                dxT = offT[:].rearrange("p (b t) -> p b t", t=2 * K2)[
                    :, :, 1:2 * K2:2].rearrange("p b t -> p (b t)")

                pidx = prep.tile([128, 1], i32, name="pidx")
                nc.gpsimd.iota(pidx[:], pattern=[[0, 1]], base=0,
                               channel_multiplier=1)
                pidxf = prep.tile([128, 1], f32, name="pidxf")
                nc.vector.tensor_copy(out=pidxf[:], in_=pidx[:])
                geh = prep.tile([128, 1], f32, name="geh")
                nc.vector.tensor_scalar(out=geh[:], in0=pidxf[:], scalar1=63.5,
                                        scalar2=None, op0=Alu.is_gt)
                gehm1 = prep.tile([128, 1], f32, name="gehm1")
                nc.vector.tensor_scalar(out=gehm1[:], in0=geh[:], scalar1=-1.0,
                                        scalar2=None, op0=Alu.add)
                blk2 = prep.tile([128, NBLK], i32, name="blk2")
                nc.gpsimd.iota(blk2[:], pattern=[[2, NBLK]], base=0,
                               channel_multiplier=0)
                ybase = prep.tile([128, NBLK], f32, name="ybase")
                nc.vector.tensor_copy(out=ybase[:], in_=blk2[:])
                nc.vector.tensor_scalar(out=ybase[:], in0=ybase[:],
                                        scalar1=gehm1[:, 0:1], scalar2=None,
                                        op0=Alu.add)
                xbase = prep.tile([128, 1], f32, name="xbase")
                nc.vector.scalar_tensor_tensor(
                    out=xbase[:], in0=geh[:], scalar=-64.0, in1=pidxf[:],
                    op0=Alu.mult, op1=Alu.add)
                nc.vector.tensor_scalar(out=xbase[:], in0=xbase[:],
                                        scalar1=-1.0, scalar2=None, op0=Alu.add)

                byk = prep.tile([128, NI], f32, name="byk")
                bxk = prep.tile([128, NI], f32, name="bxk")
                for k in range(K2):
                    nc.vector.tensor_scalar(
                        out=byk[:, k:NI:K2], in0=ybase[:],
                        scalar1=float(k // 3), scalar2=None, op0=Alu.add)
                    nc.vector.tensor_scalar(
                        out=bxk[:, k:NI:K2],
                        in0=xbase[:].broadcast_to([128, NBLK]),
                        scalar1=float(k % 3), scalar2=None, op0=Alu.add)

                py = prep.tile([128, NI], f32, name="py")
                nc.vector.tensor_tensor(out=py[:], in0=byk[:], in1=dyT,
                                        op=Alu.add)
                px = prep.tile([128, NI], f32, name="px")
                nc.vector.tensor_tensor(out=px[:], in0=bxk[:], in1=dxT,
                                        op=Alu.add)

                def frac(pos, name):
                    """-> frac f32 (pos - floor(pos)); robust to the f32->i32
                    rounding mode."""
                    ii = prep.tile([128, NI], i32, name=f"ii_{name}")
                    nc.vector.tensor_copy(out=ii[:], in_=pos[:])
                    ff = prep.tile([128, NI], f32, name=f"ff_{name}")
                    nc.vector.tensor_copy(out=ff[:], in_=ii[:])
                    gt = prep.tile([128, NI], f32, name=f"gt_{name}")
                    nc.vector.tensor_tensor(out=gt[:], in0=ff[:], in1=pos[:],
                                            op=Alu.is_gt)
                    y0 = prep.tile([128, NI], f32, name=f"y0_{name}")
                    nc.vector.tensor_tensor(out=y0[:], in0=ff[:], in1=gt[:],
                                            op=Alu.subtract)
                    fr = prep.tile([128, NI], f32, name=f"fr_{name}")
                    nc.vector.tensor_tensor(out=fr[:], in0=pos[:], in1=y0[:],
                                            op=Alu.subtract)
                    return fr

                fyf = frac(py, "y")
                fxf = frac(px, "x")
                fy16 = scalp.tile([128, NI], bf16, name="fy16")
                nc.vector.tensor_copy(out=fy16[:], in_=fyf[:])
                fx16 = scalp.tile([128, NI], bf16, name="fx16")
                nc.vector.tensor_copy(out=fx16[:], in_=fxf[:])

                # ----- gather-table build: PE-transpose 128-cell blocks of
                # each component plane into x4 rows [g, a|h|d|e], then one
                # large contiguous HWDGE write per block.
                ORDER = ("a", "h", "d", "e")
                for blk in range(NROW // 128):
                    stg = stgp.tile([128, 4 * C], bf16, tag="stg")
                    for grp in range(2):  # grp0 = [a|h], grp1 = [d|e]
                        ps = psB.tile([128, 512], bf16, tag="pstg")
                        for ci in range(2):
                            comp = ORDER[grp * 2 + ci]
                            for cc in range(2):
                                nc.tensor.transpose(
                                    out=ps[:, (ci * 2 + cc) * 128:
                                            (ci * 2 + cc) * 128 + 128],
                                    in_=comps[(comp, cc)][
                                        :, blk * 128:(blk + 1) * 128],
                                    identity=ident[:])
                        if grp == 0:
                            nc.scalar.copy(
                                out=stg[:, grp * 512:(grp + 1) * 512],
                                in_=ps[:])
                        else:
                            nc.vector.tensor_copy(
                                out=stg[:, grp * 512:(grp + 1) * 512],
                                in_=ps[:])
                    nc.scalar.dma_start(out=x4[blk * 128:(blk + 1) * 128, :],
                                        in_=stg[:])

                # ----- weights -> lhsT[c,o] bf16 tiles per (k, cchunk, ochunk)
                wT = {}
                for oc in range(2):
                    wsb = prep.tile([128, C * K2], bf16, name=f"wsb{oc}",
                                    tag="wsbb")
                    nc.gpsimd.dma_start(
                        out=wsb[:], in_=w_in[oc * 128:(oc + 1) * 128, :])
                    for k in range(K2):
                        for cc in range(2):
                            pt = psA.tile([128, 128], bf16, tag="wtp")
                            start = cc * 128 * K2 + k
                            src = wsb[:, start: start + 127 * K2 + 1: K2]
                            nc.tensor.transpose(out=pt[:], in_=src,
                                                identity=ident[:])
                            st = wpool.tile([128, 128], bf16,
                                            name=f"wT_{k}_{cc}_{oc}")
                            nc.scalar.copy(out=st[:], in_=pt[:])
                            wT[(k, cc, oc)] = st


            # ---------- main pipeline ----------
            with (
                tc.tile_pool(name="gat", bufs=6) as gatp,
                tc.tile_pool(name="spool", bufs=10) as spool,
                tc.tile_pool(name="uvpool", bufs=10) as uvpool,
                tc.tile_pool(name="stpool", bufs=2) as stpool,
                tc.tile_pool(name="pst", bufs=2, space="PSUM") as pst,
                tc.tile_pool(name="psout", bufs=2, space="PSUM") as psout,
                tc.tile_pool(name="outp", bufs=2) as outp,
            ):
                for gp in range(NBLK // MB):   # 4 iterations, 2 pgroups each
                    stiles = {}
                    for k in range(K2):
                        G = gatp.tile([128, MB, 4 * C], bf16, tag="G")
                        nc.gpsimd.dma_gather(
                            out_ap=G[:],
                            in_ap=x4[:, :],
                            idxs_ap=idx16[:, (k * 4 + gp) * 64:
                                          (k * 4 + gp) * 64 + 64],
                            num_idxs=MB * 128,
                            num_idxs_reg=MB * 128,
                            elem_size=4 * C,
                            single_packet=False,
                            queue_num=(gp * K2 + k) % 4)
                        for half in range(2):
                            ps = {cc: pst.tile([128, 512], bf16,
                                               tag=f"stp{cc}", name=f"stp{cc}")
                                  for cc in range(2)}
                            Ss = []
                            for m4 in range(4):
                                m = half * 4 + m4
                                col = (gp * MB + m) * K2 + k
                                # UV = [a|h] + fy * [d|e]   (free dim 512)
                                UV = uvpool.tile([128, 2 * C], bf16, tag="UV",
                                                 name="UV")
                                nc.vector.scalar_tensor_tensor(
                                    out=UV[:], in0=G[:, m, 2 * C:4 * C],
                                    scalar=fy16[:, col:col + 1],
                                    in1=G[:, m, 0:2 * C],
                                    op0=Alu.mult, op1=Alu.add)
                                # S = u + fx * v            (free dim 256)
                                S = spool.tile([128, C], bf16, tag="S",
                                               name="S")
                                nc.vector.scalar_tensor_tensor(
                                    out=S[:], in0=UV[:, C:2 * C],
                                    scalar=fx16[:, col:col + 1],
                                    in1=UV[:, 0:C],
                                    op0=Alu.mult, op1=Alu.add)
                                Ss.append(S)
                            for m4 in range(4):
                                for cc in range(2):
                                    nc.tensor.transpose(
                                        out=ps[cc][:, m4 * 128:(m4 + 1) * 128],
                                        in_=Ss[m4][:, cc * 128:(cc + 1) * 128],
                                        identity=ident[:])
                            for cc in range(2):
                                st = stpool.tile([128, 512], bf16,
                                                 tag=f"st_{k}_{cc}", bufs=2)
                                nc.scalar.copy(out=st[:], in_=ps[cc][:])
                                stiles[(half, k, cc)] = st

                    for half in range(2):
                        pg = gp * 2 + half
                        for oc in range(2):
                            acc = psout.tile([128, 512], f32, tag="acc")
                            first = True
                            for k in range(K2):
                                for cc in range(2):
                                    nc.tensor.matmul(
                                        out=acc[:],
                                        lhsT=wT[(k, cc, oc)][:],
                                        rhs=stiles[(half, k, cc)][:],
                                        start=first,
                                        stop=(k == K2 - 1 and cc == 1))
                                    first = False
                            osb = outp.tile([128, 512], f32, tag="osb")
                            nc.scalar.copy(out=osb[:], in_=acc[:])
                            nc.sync.dma_start(
                                out=out_ext[oc * 128:(oc + 1) * 128,
                                            pg * 512:(pg + 1) * 512],
                                in_=osb[:])

    nc.compile()
    return nc


def kernel(x, offset, weight):
    global _BUILT
    from concourse import bass_utils

    if _BUILT is None:
        _BUILT = _build_kernel()
    nc = _BUILT

    B = x.shape[0]
    x = np.ascontiguousarray(np.asarray(x, np.float32).reshape(B, C, HW))
    offset = np.ascontiguousarray(
        np.asarray(offset, np.float32).reshape(B, 2 * K2, HW))
    weight = np.ascontiguousarray(
        np.asarray(weight, np.float32).reshape(O, C * K2))

    in_maps = [{"x": x[b], "offset": offset[b], "weight": weight}
               for b in range(B)]
    res = bass_utils.run_bass_kernel_spmd(nc, in_maps, core_ids=list(range(B)))
    outs = [np.asarray(res.results[b]["out"]).reshape(O, H, W)
            for b in range(B)]
    return np.stack(outs).astype(np.float32)
